# revision 14
# baseline (speedup 1.0000x reference)
"""Trainium2 Bass kernel for nn_CrossExpertRefinement.

Data-parallel over B=8 (one batch row per NeuronCore). Per core:
feature-major layout [feat, tok], 4 token blocks of 512, per-expert
streaming. fp32r matmuls on the residual backbone (Win, Wqkv, Wo, Wout,
identity-x), bf16 only for the q*k score products. LayerNorm mean is
folded into the Win matmul (extra mean-weight column); LN gamma/beta and
the 1/sqrt(HD) score scale are folded into Wqkv host-side; the x residual
is an identity matmul accumulated in PSUM.
"""
import numpy as np
import ml_dtypes
from contextlib import ExitStack

import concourse.bass as bass
import concourse.bacc as bacc
import concourse.tile as tile
from concourse import mybir
from concourse.bass_utils import run_bass_kernel_spmd

dt = mybir.dt
BF = ml_dtypes.bfloat16

D, H, HD, NE = 512, 8, 64, 4
EXPERT_DIMS = [128, 128, 1006, 104]
DPAD = [128, 128, 1024, 128]
KCH = [p // 128 for p in DPAD]
LN_EPS = 1e-5
B, T = 8, 2048
NCORES = 8


def _bc(ap, pos, count):
    """Insert a step-0 (broadcast) dim into an AP at free-dim position pos
    (0 = right after the partition dim)."""
    new = list(ap.ap)
    new.insert(1 + pos, [0, count])
    return bass.AP(tensor=ap.tensor, offset=ap.offset, ap=new)


def build_program(tcore=T, tb=512, qkv_bias=False, o_bias=False):
    f32, f32r, bf16 = dt.float32, dt.float32r, dt.bfloat16
    nb = tcore // tb
    nc = bacc.Bacc()

    xT = [nc.dram_tensor(f"xT{i}", [DPAD[i], tcore], f32r, kind="ExternalInput")
          for i in range(NE)]
    winT = [nc.dram_tensor(f"winT{i}", [DPAD[i], 514], f32r, kind="ExternalInput")
            for i in range(NE)]
    wqkvT_d = nc.dram_tensor("wqkvT", [D, 3 * D], f32r, kind="ExternalInput")
    woT_d = nc.dram_tensor("woT", [D, D], f32r, kind="ExternalInput")
    woutT = [nc.dram_tensor(f"woutT{i}", [D, DPAD[i]], f32r, kind="ExternalInput")
             for i in range(NE)]
    i128_d = nc.dram_tensor("i128", [128, 128], f32r, kind="ExternalInput")
    bd_d = nc.dram_tensor("bd", [128, 4 * 8], bf16, kind="ExternalInput")
    ebc_d = nc.dram_tensor("ebc", [8, 4 * 128], f32r, kind="ExternalInput")
    onesm_d = nc.dram_tensor("onesm", [128, 1], bf16, kind="ExternalInput")
    onesk_d = nc.dram_tensor("onesk", [1, 128], f32r, kind="ExternalInput")
    if qkv_bias:
        bq_d = nc.dram_tensor("bqkv_eff", [1, 3 * D], f32r, kind="ExternalInput")
    if o_bias:
        bo_d = nc.dram_tensor("bo_eff", [1, D], f32r, kind="ExternalInput")
    outT = [nc.dram_tensor(f"outT{i}", [EXPERT_DIMS[i], tcore], f32,
                           kind="ExternalOutput") for i in range(NE)]

    with tile.TileContext(nc) as tc, ExitStack() as ctx:
        wsb = ctx.enter_context(tc.tile_pool(name="wsb", bufs=1))
        xp = ctx.enter_context(tc.tile_pool(name="xp", bufs=1))
        xr = ctx.enter_context(tc.tile_pool(name="xr", bufs=1))
        sp = ctx.enter_context(tc.tile_pool(name="sp", bufs=1))
        zc = ctx.enter_context(tc.tile_pool(name="zc", bufs=1))
        sc16 = ctx.enter_context(tc.tile_pool(name="sc16", bufs=1))
        kvp = ctx.enter_context(tc.tile_pool(name="kvp", bufs=1))
        qp = ctx.enter_context(tc.tile_pool(name="qp", bufs=1))
        op_ = ctx.enter_context(tc.tile_pool(name="op", bufs=2))
        sm = ctx.enter_context(tc.tile_pool(name="sm", bufs=1))
        tm = ctx.enter_context(tc.tile_pool(name="tm", bufs=1))
        pbig = ctx.enter_context(tc.tile_pool(name="pbig", bufs=2, space="PSUM"))
        ptin = ctx.enter_context(tc.tile_pool(name="ptin", bufs=1, space="PSUM"))

        # ---- weights (loaded once) ----
        win_sb = []
        for i in range(NE):
            t = wsb.tile([128, KCH[i], 514], f32r, tag=f"win{i}")
            nc.sync.dma_start(out=t, in_=xT_rearr(winT[i], KCH[i], 513))
            win_sb.append(t)
        wqkv_sb = wsb.tile([128, 4, 3 * D], f32r, tag="wqkv")
        nc.sync.dma_start(out=wqkv_sb, in_=xT_rearr(wqkvT_d, 4, 3 * D))
        wo_sb = wsb.tile([128, 4, D], f32r, tag="wo")
        nc.sync.dma_start(out=wo_sb, in_=xT_rearr(woT_d, 4, D))
        wout_sb = []
        for i in range(NE):
            t = wsb.tile([128, 4, DPAD[i]], f32r, tag=f"wout{i}")
            nc.sync.dma_start(out=t, in_=xT_rearr(woutT[i], 4, DPAD[i]))
            wout_sb.append(t)
        i128_sb = wsb.tile([128, 128], f32r, tag="i128")
        nc.sync.dma_start(out=i128_sb, in_=i128_d[:, :])
        bd_sb = wsb.tile([128, 4, 8], bf16, tag="bd")
        nc.sync.dma_start(out=bd_sb, in_=bd_d.rearrange("p (c h) -> p c h", c=4))
        ebc_sb = wsb.tile([8, 4, 128], f32r, tag="ebc")
        nc.sync.dma_start(out=ebc_sb, in_=ebc_d.rearrange("h (c p) -> h c p", c=4))
        onesm_sb = wsb.tile([128, 1], bf16, tag="onesm")
        nc.sync.dma_start(out=onesm_sb, in_=onesm_d[:, :])
        onesk_sb = wsb.tile([1, 128], f32r, tag="onesk")
        nc.sync.dma_start(out=onesk_sb, in_=onesk_d[:, :])
        if qkv_bias:
            bq_sb = wsb.tile([1, 3 * D], f32r, tag="bq")
            nc.sync.dma_start(out=bq_sb, in_=bq_d[:, :])
        if o_bias:
            bo_sb = wsb.tile([1, D], f32r, tag="bo")
            nc.sync.dma_start(out=bo_sb, in_=bo_d[:, :])
        ones_row = wsb.tile([1, tb], f32r, tag="ones_row")
        if qkv_bias or o_bias:
            nc.gpsimd.memset(ones_row[:, :], 1.0)
        eps_sb = wsb.tile([1, 1], f32, tag="eps")
        nc.gpsimd.memset(eps_sb[:, :], LN_EPS)

        for b in range(nb):
            t0 = b * tb
            kb = kvp.tile([128, NE, 4, tb], bf16, tag="kb")    # [p, j, c, t]
            vb = kvp.tile([128, NE, 4, tb], bf16, tag="vb")
            for i in range(NE):
                kc = KCH[i]
                # ---- stage A: input projection + LN stats ----
                s_lo = pbig.tile([128, 2, tb], f32, tag="pb")
                s_hi = pbig.tile([128, 2, tb], f32, tag="pb")
                pt = ptin.tile([128, 4, tb], f32, tag="pt")
                for c0 in range(0, kc, 4):
                    cn = min(4, kc - c0)
                    xt = xp.tile([128, 4, tb], f32r, tag="x")
                    src = xT[i][128 * c0:128 * (c0 + cn), t0:t0 + tb].rearrange(
                        "(c p) t -> p c t", p=128)
                    nc.sync.dma_start(out=xt[:, 0:cn, :], in_=src)
                    for m in range(4):
                        ps = (s_lo, s_hi)[m // 2]
                        for cl in range(cn):
                            c = c0 + cl
                            nc.tensor.matmul(ps[:, m % 2, :],
                                             win_sb[i][:, c, m * 128:(m + 1) * 128],
                                             xt[:, cl, :],
                                             start=(c == 0), stop=(c == kc - 1),
                                             skip_group_check=True)
                    for cl in range(cn):
                        c = c0 + cl
                        nc.tensor.matmul(pt[0:2, 0, :], win_sb[i][:, c, 512:514],
                                         xt[:, cl, :], start=(c == 0),
                                         stop=(c == kc - 1),
                                         skip_group_check=True)

                s_sb = sp.tile([128, 4, tb], f32r, tag="S")
                s2 = sc16.tile([128, 4, tb], bf16, tag="sc16")
                for hf, ps in ((0, s_lo), (1, s_hi)):
                    nc.scalar.copy(out=s_sb[:, 2 * hf:2 * hf + 2, :], in_=ps[:, :, :])
                    nc.scalar.square(out=s2[:, 2 * hf:2 * hf + 2, :], in_=ps[:, :, :])
                for c in range(4):
                    nc.tensor.matmul(pt[64:65, 0, :], onesm_sb[:, :], s2[:, c, :],
                                     start=(c == 0), stop=(c == 3))

                # ---- stats -> inv, mu*inv ----
                st = sm.tile([1, 4, tb], f32, tag="stats")
                nc.scalar.copy(out=st[:, 0, :], in_=pt[0:1, 0, :])        # mu
                nc.scalar.copy(out=st[:, 1, :], in_=pt[64:65, 0, :])      # E[S^2]
                nc.vector.scalar_tensor_tensor(out=st[:, 2, :], in0=st[:, 0, :],
                                               scalar=1.0, in1=st[:, 0, :],
                                               op0=mybir.AluOpType.mult,
                                               op1=mybir.AluOpType.mult)  # mu^2
                nc.vector.tensor_sub(st[:, 3, :], st[:, 1, :], st[:, 2, :])  # var
                nc.scalar.activation(out=st[:, 2, :], in_=st[:, 3, :],
                                     func=mybir.ActivationFunctionType.Sqrt,
                                     bias=eps_sb[0:1, 0:1])               # std
                inv_t = sm.tile([1, tb], f32r, tag="inv")
                miv_t = sm.tile([1, tb], f32r, tag="miv")
                with nc.allow_low_precision(reason="f32r is 32-bit"):
                    nc.vector.reciprocal(out=inv_t[0:1, :], in_=st[:, 2, :])  # inv
                nc.vector.tensor_mul(miv_t[0:1, :], st[:, 0, :], inv_t[0:1, :])

                binv = pbig.tile([128, 2, tb], f32, tag="pb")
                nc.tensor.matmul(binv[:, 0, :], onesk_sb[:, :], inv_t[0:1, :],
                                 start=True, stop=True)
                nc.tensor.matmul(binv[:, 1, :], onesk_sb[:, :], miv_t[0:1, :],
                                 start=True, stop=True)

                zt = zc.tile([128, 4, tb], f32r, tag="zc")
                nc.vector.tensor_mul(zt[:, :, :], s_sb[:, :, :],
                                     _bc(binv[:, 0, :], 0, 4))
                nc.vector.tensor_sub(zt[:, :, :], zt[:, :, :],
                                     _bc(binv[:, 1, :], 0, 4))

                # ---- stage B: qkv ----
                groups = [] if i == 0 else [(0, "q")]
                groups += [(1, "k"), (2, "v")]
                qb = None
                for gi, gname in groups:
                    for half in range(2):
                        mms = [gi * 4 + 2 * half, gi * 4 + 2 * half + 1]
                        gps = pbig.tile([128, 2, tb], f32, tag="pb")
                        for mi, m in enumerate(mms):
                            for c in range(4):
                                nc.tensor.matmul(
                                    gps[:, mi, :],
                                    wqkv_sb[:, c, m * 128:(m + 1) * 128],
                                    zt[:, c, :],
                                    start=(c == 0),
                                    stop=(c == 3 and not qkv_bias))
                            if qkv_bias:
                                nc.tensor.matmul(
                                    gps[:, mi, :],
                                    bq_sb[0:1, m * 128:(m + 1) * 128],
                                    ones_row[:, :], start=False, stop=True)
                        if gname == "q":
                            if qb is None:
                                qb = qp.tile([128, 4, tb], bf16, tag="qb")
                            nc.scalar.copy(out=qb[:, 2 * half:2 * half + 2, :],
                                           in_=gps[:, :, :])
                        elif gname == "k":
                            nc.scalar.copy(out=kb[:, i, 2 * half:2 * half + 2, :],
                                           in_=gps[:, :, :])
                        else:
                            nc.scalar.copy(out=vb[:, i, 2 * half:2 * half + 2, :],
                                           in_=gps[:, :, :])

                # ---- stage C: scores + softmax (i > 0) ----
                if i > 0:
                    nj = i + 1
                    for c in range(4):
                        pmul = sc16.tile([128, 4, tb], bf16, tag="sc16")
                        nc.vector.tensor_mul(pmul[:, :nj, :],
                                             _bc(qb[:, c, :], 0, nj),
                                             kb[:, 0:nj, c, :])
                        for j in range(nj):
                            nc.tensor.matmul(pt[0:8, j, :], bd_sb[:, c, :],
                                             pmul[:, j, :],
                                             start=(c == 0), stop=(c == 3),
                                             skip_group_check=True)
                    attn = sm.tile([8, 4, tb], f32r, tag="attn")
                    nc.scalar.copy(out=attn[:, 0:nj, :], in_=pt[0:8, 0:nj, :])
                    mx = sm.tile([8, tb], f32, tag="mx")
                    nc.vector.tensor_reduce(
                        out=mx[:, :], in_=attn[:, 0:nj, :].rearrange("p j t -> p t j"),
                        axis=mybir.AxisListType.X, op=mybir.AluOpType.max)
                    nc.vector.tensor_sub(attn[:, 0:nj, :], attn[:, 0:nj, :],
                                         _bc(mx[:, :], 0, nj))
                    nc.scalar.activation(out=attn[:, 0:nj, :], in_=attn[:, 0:nj, :],
                                         func=mybir.ActivationFunctionType.Exp)
                    smsum = sm.tile([8, tb], f32, tag="smsum")
                    nc.vector.tensor_reduce(
                        out=smsum[:, :],
                        in_=attn[:, 0:nj, :].rearrange("p j t -> p t j"),
                        axis=mybir.AxisListType.X, op=mybir.AluOpType.add)
                    with nc.allow_low_precision(reason="recip of fp32 sum"):
                        nc.vector.reciprocal(out=smsum[:, :], in_=smsum[:, :])
                    nc.vector.tensor_mul(attn[:, 0:nj, :], attn[:, 0:nj, :],
                                         _bc(smsum[:, :], 0, nj))

                # ---- stage D: ctx + Wo ----
                if i == 0:
                    ctxt = zc.tile([128, 4, tb], f32r, tag="zc")
                    nc.vector.tensor_copy(ctxt[:, :, :], vb[:, 0, :, :])
                else:
                    nj = i + 1
                    ctxt = zc.tile([128, 4, tb], f32r, tag="zc")
                    for c in range(4):
                        npair = (nj + 1) // 2
                        abt = []
                        for half in range(npair):
                            at = pbig.tile([128, 2, tb], f32, tag="pb")
                            for jj in range(2):
                                j = half * 2 + jj
                                if j < nj:
                                    nc.tensor.matmul(at[:, jj, :],
                                                     ebc_sb[:, c, :],
                                                     attn[:, j, :],
                                                     start=True, stop=True)
                            abt.append(at)
                        tmp = tm.tile([128, 4, tb], f32, tag="tmp32")
                        for half in range(npair):
                            jn = min(2, nj - half * 2)
                            nc.vector.tensor_mul(
                                tmp[:, half * 2:half * 2 + jn, :],
                                abt[half][:, 0:jn, :],
                                vb[:, half * 2:half * 2 + jn, c, :])
                        nc.vector.tensor_copy(ctxt[:, c, :], tmp[:, 0, :])
                        for j in range(1, nj):
                            nc.vector.tensor_add(ctxt[:, c, :], ctxt[:, c, :],
                                                 tmp[:, j, :])

                r_lo = pbig.tile([128, 2, tb], f32, tag="pb")
                r_hi = pbig.tile([128, 2, tb], f32, tag="pb")
                for m in range(4):
                    ps = (r_lo, r_hi)[m // 2]
                    for c in range(4):
                        nc.tensor.matmul(ps[:, m % 2, :],
                                         wo_sb[:, c, m * 128:(m + 1) * 128],
                                         ctxt[:, c, :], start=(c == 0),
                                         stop=(c == 3 and not o_bias))
                    if o_bias:
                        nc.tensor.matmul(ps[:, m % 2, :],
                                         bo_sb[0:1, m * 128:(m + 1) * 128],
                                         ones_row[:, :], start=False, stop=True)
                # stacked' = S + refined  (in place on s_sb)
                nc.vector.tensor_add(s_sb[:, 0:2, :], s_sb[:, 0:2, :], r_lo[:, :, :])
                nc.vector.tensor_add(s_sb[:, 2:4, :], s_sb[:, 2:4, :], r_hi[:, :, :])

                # ---- stage E: output projection + x residual ----
                mch = DPAD[i] // 128
                for mp in range(0, mch, 2):
                    mn = min(2, mch - mp)
                    ops = pbig.tile([128, 2, tb], f32, tag="pb")
                    xres = xr.tile([128, 2, tb], f32r, tag="xres")
                    src = xT[i][128 * mp:128 * (mp + mn), t0:t0 + tb].rearrange(
                        "(c p) t -> p c t", p=128)
                    nc.sync.dma_start(out=xres[:, 0:mn, :], in_=src)
                    for mi in range(mn):
                        m = mp + mi
                        for c in range(4):
                            nc.tensor.matmul(ops[:, mi, :],
                                             wout_sb[i][:, c, m * 128:(m + 1) * 128],
                                             s_sb[:, c, :],
                                             start=(c == 0), stop=False)
                        nc.tensor.matmul(ops[:, mi, :], i128_sb[:, :],
                                         xres[:, mi, :], start=False, stop=True)
                    for mi in range(mn):
                        m = mp + mi
                        rows = min(128, EXPERT_DIMS[i] - 128 * m)
                        if rows <= 0:
                            continue
                        osb = op_.tile([128, tb], f32, tag="osb")
                        nc.scalar.copy(out=osb[:, :], in_=ops[:, mi, :])
                        nc.sync.dma_start(
                            out=outT[i][128 * m:128 * m + rows, t0:t0 + tb],
                            in_=osb[:rows, :])
    nc.compile()
    return nc


def xT_rearr(handle, kc, m):
    return handle[:, :].rearrange("(c p) m -> p c m", p=128)


def prep_shared(inputs):
    """Host-side preprocessing of weights/constants (shared by all cores)."""
    f32 = np.float32
    g = np.asarray(inputs["ln_g"], f32)
    beta = np.asarray(inputs["ln_b"], f32)
    Wqkv = np.asarray(inputs["Wqkv"], f32)
    bqkv = np.asarray(inputs["bqkv"], f32)
    Wo = np.asarray(inputs["Wo"], f32)
    bo = np.asarray(inputs["bo"], f32)

    Wq_eff = Wqkv * g[None, :]
    bq_eff = bqkv + Wqkv @ beta
    Wq_eff[:D] *= 1.0 / np.sqrt(HD)
    bq_eff[:D] *= 1.0 / np.sqrt(HD)

    out = {}
    for i in range(NE):
        d, dp = EXPERT_DIMS[i], DPAD[i]
        Win = np.asarray(inputs[f"Win{i}"], f32)          # (D, d)
        w = np.zeros((dp, 514), f32)
        w[:d, :D] = Win.T
        w[:d, 512] = Win.mean(axis=0)
        out[f"winT{i}"] = w
        Wout = np.asarray(inputs[f"Wout{i}"], f32)        # (d, D)
        wo_ = np.zeros((D, dp), f32)
        wo_[:, :d] = Wout.T
        out[f"woutT{i}"] = wo_
    out["wqkvT"] = np.ascontiguousarray(Wq_eff.T)         # (D, 3D)
    out["woT"] = np.ascontiguousarray(Wo.T)
    out["i128"] = np.eye(128, dtype=f32)
    bd = np.zeros((128, 4, 8), f32)
    for c in range(4):
        for p in range(128):
            bd[p, c, 2 * c + p // 64] = 1.0
    out["bd"] = bd.reshape(128, 32).astype(BF)
    ebc = np.zeros((8, 4, 128), f32)
    for c in range(4):
        for p in range(128):
            ebc[2 * c + p // 64, c, p] = 1.0
    out["ebc"] = np.ascontiguousarray(ebc.reshape(8, 512))
    out["onesm"] = np.full((128, 1), 1.0 / 512.0, f32).astype(BF)
    out["onesk"] = np.ones((1, 128), f32)
    qkv_bias = bool(np.abs(bq_eff).max() > 0)
    o_bias = bool(np.abs(bo).max() > 0)
    if qkv_bias:
        out["bqkv_eff"] = bq_eff.reshape(1, 3 * D)
    if o_bias:
        out["bo_eff"] = bo.reshape(1, D)
    return out, qkv_bias, o_bias


def prep_core_inputs(inputs, core, shared, tcore=T):
    """Per-core inputs: transposed, padded x slices."""
    m = dict(shared)
    for i in range(NE):
        d, dp = EXPERT_DIMS[i], DPAD[i]
        x = np.asarray(inputs[f"x{i}"], np.float32)[core]      # (T, d)
        xt = np.zeros((dp, tcore), np.float32)
        xt[:d, :] = x.T
        m[f"xT{i}"] = xt
    return m


_cached = {}


def kernel(**inputs):
    shared, qkv_bias, o_bias = prep_shared(inputs)
    key = (T, 512, qkv_bias, o_bias)
    if key not in _cached:
        _cached[key] = build_program(T, 512, qkv_bias, o_bias)
    nc = _cached[key]
    in_maps = [prep_core_inputs(inputs, c, shared) for c in range(NCORES)]
    res = run_bass_kernel_spmd(nc, in_maps, list(range(NCORES)))
    outs = []
    for i in range(NE):
        full = np.empty((B, T, EXPERT_DIMS[i]), np.float32)
        for c in range(NCORES):
            full[c] = res.results[c][f"outT{i}"].T
        outs.append(full)
    return tuple(outs)


# revision 19
# speedup vs baseline: 1.0581x; 1.0581x over previous
"""Trainium2 Bass kernel for nn_CrossExpertRefinement.

Data-parallel over B=8 (one batch row per NeuronCore). Per core:
feature-major layout [feat, tok], 4 token blocks of 512, per-expert
streaming. fp32r matmuls on the residual backbone (Win, Wqkv, Wo, Wout,
identity-x), bf16 only for the q*k score products. LayerNorm mean is
folded into the Win matmul (extra mean-weight column); LN gamma/beta and
the 1/sqrt(HD) score scale are folded into Wqkv host-side; the x residual
is an identity matmul accumulated in PSUM.
"""
import numpy as np
import ml_dtypes
from contextlib import ExitStack

import concourse.bass as bass
import concourse.bacc as bacc
import concourse.tile as tile
from concourse import mybir
from concourse.bass_utils import run_bass_kernel_spmd

dt = mybir.dt
BF = ml_dtypes.bfloat16

D, H, HD, NE = 512, 8, 64, 4
EXPERT_DIMS = [128, 128, 1006, 104]
DPAD = [128, 128, 1024, 128]
KCH = [p // 128 for p in DPAD]
LN_EPS = 1e-5
B, T = 8, 2048
NCORES = 8


def _bc(ap, pos, count):
    """Insert a step-0 (broadcast) dim into an AP at free-dim position pos
    (0 = right after the partition dim)."""
    new = list(ap.ap)
    new.insert(1 + pos, [0, count])
    return bass.AP(tensor=ap.tensor, offset=ap.offset, ap=new)


def build_program(tcore=T, tb=512, qkv_bias=False, o_bias=False):
    f32, f32r, bf16 = dt.float32, dt.float32r, dt.bfloat16
    nb = tcore // tb
    nc = bacc.Bacc()

    xT = [nc.dram_tensor(f"xT{i}", [DPAD[i], tcore], f32r, kind="ExternalInput")
          for i in range(NE)]
    winT = [nc.dram_tensor(f"winT{i}", [DPAD[i], 514], f32r, kind="ExternalInput")
            for i in range(NE)]
    wqkvT_d = nc.dram_tensor("wqkvT", [D, 3 * D], f32r, kind="ExternalInput")
    woT_d = nc.dram_tensor("woT", [D, D], f32r, kind="ExternalInput")
    woutT = [nc.dram_tensor(f"woutT{i}", [D, DPAD[i]], f32r, kind="ExternalInput")
             for i in range(NE)]
    i128_d = nc.dram_tensor("i128", [128, 128], f32r, kind="ExternalInput")
    bd_d = nc.dram_tensor("bd", [128, 4 * 8], bf16, kind="ExternalInput")
    ebc_d = nc.dram_tensor("ebc", [8, 4 * 128], f32r, kind="ExternalInput")
    onesm_d = nc.dram_tensor("onesm", [128, 1], bf16, kind="ExternalInput")
    onesk_d = nc.dram_tensor("onesk", [1, 128], f32r, kind="ExternalInput")
    if qkv_bias:
        bq_d = nc.dram_tensor("bqkv_eff", [1, 3 * D], f32r, kind="ExternalInput")
    if o_bias:
        bo_d = nc.dram_tensor("bo_eff", [1, D], f32r, kind="ExternalInput")
    outT = [nc.dram_tensor(f"outT{i}", [EXPERT_DIMS[i], tcore], f32,
                           kind="ExternalOutput") for i in range(NE)]

    with tile.TileContext(nc) as tc, ExitStack() as ctx:
        wsb = ctx.enter_context(tc.tile_pool(name="wsb", bufs=1))
        xp = ctx.enter_context(tc.tile_pool(name="xp", bufs=2))
        xr = ctx.enter_context(tc.tile_pool(name="xr", bufs=1))
        sp = ctx.enter_context(tc.tile_pool(name="sp", bufs=2))
        zc = ctx.enter_context(tc.tile_pool(name="zc", bufs=1))
        sc16 = ctx.enter_context(tc.tile_pool(name="sc16", bufs=2))
        kvp = ctx.enter_context(tc.tile_pool(name="kvp", bufs=1))
        qp = ctx.enter_context(tc.tile_pool(name="qp", bufs=1))
        op_ = ctx.enter_context(tc.tile_pool(name="op", bufs=2))
        sm = ctx.enter_context(tc.tile_pool(name="sm", bufs=1))
        tm = ctx.enter_context(tc.tile_pool(name="tm", bufs=1))
        pbig = ctx.enter_context(tc.tile_pool(name="pbig", bufs=2, space="PSUM"))
        ptin = ctx.enter_context(tc.tile_pool(name="ptin", bufs=4, space="PSUM"))

        # ---- weights (loaded once) ----
        win_sb = []
        for i in range(NE):
            t = wsb.tile([128, KCH[i], 514], f32r, tag=f"win{i}")
            nc.sync.dma_start(out=t, in_=xT_rearr(winT[i], KCH[i], 513))
            win_sb.append(t)
        wqkv_sb = wsb.tile([128, 4, 3 * D], f32r, tag="wqkv")
        nc.sync.dma_start(out=wqkv_sb, in_=xT_rearr(wqkvT_d, 4, 3 * D))
        wo_sb = wsb.tile([128, 4, D], f32r, tag="wo")
        nc.sync.dma_start(out=wo_sb, in_=xT_rearr(woT_d, 4, D))
        wout_sb = []
        for i in range(NE):
            t = wsb.tile([128, 4, DPAD[i]], f32r, tag=f"wout{i}")
            nc.sync.dma_start(out=t, in_=xT_rearr(woutT[i], 4, DPAD[i]))
            wout_sb.append(t)
        i128_sb = wsb.tile([128, 128], f32r, tag="i128")
        nc.sync.dma_start(out=i128_sb, in_=i128_d[:, :])
        bd_sb = wsb.tile([128, 4, 8], bf16, tag="bd")
        nc.sync.dma_start(out=bd_sb, in_=bd_d.rearrange("p (c h) -> p c h", c=4))
        ebc_sb = wsb.tile([8, 4, 128], f32r, tag="ebc")
        nc.sync.dma_start(out=ebc_sb, in_=ebc_d.rearrange("h (c p) -> h c p", c=4))
        onesm_sb = wsb.tile([128, 1], bf16, tag="onesm")
        nc.sync.dma_start(out=onesm_sb, in_=onesm_d[:, :])
        onesk_sb = wsb.tile([1, 128], f32r, tag="onesk")
        nc.sync.dma_start(out=onesk_sb, in_=onesk_d[:, :])
        if qkv_bias:
            bq_sb = wsb.tile([1, 3 * D], f32r, tag="bq")
            nc.sync.dma_start(out=bq_sb, in_=bq_d[:, :])
        if o_bias:
            bo_sb = wsb.tile([1, D], f32r, tag="bo")
            nc.sync.dma_start(out=bo_sb, in_=bo_d[:, :])
        if qkv_bias or o_bias:
            ones_row = wsb.tile([1, tb], f32r, tag="ones_row")
            nc.gpsimd.memset(ones_row[:, :], 1.0)
        eps_sb = wsb.tile([1, 1], f32, tag="eps")
        nc.gpsimd.memset(eps_sb[:, :], LN_EPS)

        for b in range(nb):
            t0 = b * tb
            kb = kvp.tile([128, NE, 4, tb], bf16, tag="kb")    # [p, j, c, t]
            vb = kvp.tile([128, NE, 4, tb], bf16, tag="vb")
            for i in range(NE):
                kc = KCH[i]
                # ---- stage A: input projection + LN stats ----
                s_lo = pbig.tile([128, 2, tb], f32, tag="pb")
                s_hi = pbig.tile([128, 2, tb], f32, tag="pb")
                spt = ptin.tile([128, tb], f32, tag="pt")
                for c0 in range(0, kc, 4):
                    cn = min(4, kc - c0)
                    xt = xp.tile([128, 4, tb], f32r, tag="x")
                    src = xT[i][128 * c0:128 * (c0 + cn), t0:t0 + tb].rearrange(
                        "(c p) t -> p c t", p=128)
                    nc.sync.dma_start(out=xt[:, 0:cn, :], in_=src)
                    for m in range(4):
                        ps = (s_lo, s_hi)[m // 2]
                        for cl in range(cn):
                            c = c0 + cl
                            nc.tensor.matmul(ps[:, m % 2, :],
                                             win_sb[i][:, c, m * 128:(m + 1) * 128],
                                             xt[:, cl, :],
                                             start=(c == 0), stop=(c == kc - 1),
                                             skip_group_check=True)
                    for cl in range(cn):
                        c = c0 + cl
                        nc.tensor.matmul(spt[0:2, :], win_sb[i][:, c, 512:514],
                                         xt[:, cl, :], start=(c == 0),
                                         stop=(c == kc - 1),
                                         skip_group_check=True)

                s_sb = sp.tile([128, 4, tb], f32r, tag="S")
                s2 = sc16.tile([128, 4, tb], bf16, tag="sc16")
                for hf, ps in ((0, s_lo), (1, s_hi)):
                    nc.scalar.copy(out=s_sb[:, 2 * hf:2 * hf + 2, :], in_=ps[:, :, :])
                    nc.scalar.square(out=s2[:, 2 * hf:2 * hf + 2, :], in_=ps[:, :, :])
                for c in range(4):
                    nc.tensor.matmul(spt[64:65, :], onesm_sb[:, :], s2[:, c, :],
                                     start=(c == 0), stop=(c == 3))

                # ---- stats -> inv, mu*inv ----
                st = sm.tile([1, 4, tb], f32, tag="stats")
                nc.scalar.copy(out=st[:, 0, :], in_=spt[0:1, :])          # mu
                nc.scalar.copy(out=st[:, 1, :], in_=spt[64:65, :])        # E[S^2]
                nc.gpsimd.tensor_mul(st[:, 2, :], st[:, 0, :], st[:, 0, :])  # mu^2
                nc.gpsimd.tensor_sub(st[:, 3, :], st[:, 1, :], st[:, 2, :])  # var
                # inv = rsqrt(var + eps) = exp(-0.5 * ln(var + eps))
                nc.scalar.activation(out=st[:, 2, :], in_=st[:, 3, :],
                                     func=mybir.ActivationFunctionType.Ln,
                                     bias=eps_sb[0:1, 0:1])
                inv_t = sm.tile([1, tb], f32r, tag="inv")
                miv_t = sm.tile([1, tb], f32r, tag="miv")
                with nc.allow_low_precision(reason="f32r is 32-bit"):
                    nc.scalar.activation(out=inv_t[0:1, :], in_=st[:, 2, :],
                                         func=mybir.ActivationFunctionType.Exp,
                                         scale=-0.5)
                nc.gpsimd.tensor_mul(miv_t[0:1, :], st[:, 0, :], inv_t[0:1, :])

                inv_b = ptin.tile([128, tb], f32, tag="pt")
                miv_b = ptin.tile([128, tb], f32, tag="pt")
                nc.tensor.matmul(inv_b[:, :], onesk_sb[:, :], inv_t[0:1, :],
                                 start=True, stop=True)
                nc.tensor.matmul(miv_b[:, :], onesk_sb[:, :], miv_t[0:1, :],
                                 start=True, stop=True)

                zt = zc.tile([128, 4, tb], f32r, tag="zc")
                nc.vector.tensor_mul(zt[:, :, :], s_sb[:, :, :],
                                     _bc(inv_b[:, :], 0, 4))
                nc.vector.tensor_sub(zt[:, :, :], zt[:, :, :],
                                     _bc(miv_b[:, :], 0, 4))

                # ---- stage B: qkv ----
                groups = [] if i == 0 else [(0, "q")]
                groups += [(1, "k"), (2, "v")]
                qb = None
                for gi, gname in groups:
                    for half in range(2):
                        mms = [gi * 4 + 2 * half, gi * 4 + 2 * half + 1]
                        gps = pbig.tile([128, 2, tb], f32, tag="pb")
                        for mi, m in enumerate(mms):
                            for c in range(4):
                                nc.tensor.matmul(
                                    gps[:, mi, :],
                                    wqkv_sb[:, c, m * 128:(m + 1) * 128],
                                    zt[:, c, :],
                                    start=(c == 0),
                                    stop=(c == 3 and not qkv_bias))
                            if qkv_bias:
                                nc.tensor.matmul(
                                    gps[:, mi, :],
                                    bq_sb[0:1, m * 128:(m + 1) * 128],
                                    ones_row[:, :], start=False, stop=True)
                        if gname == "q":
                            if qb is None:
                                qb = qp.tile([128, 4, tb], bf16, tag="qb")
                            nc.scalar.copy(out=qb[:, 2 * half:2 * half + 2, :],
                                           in_=gps[:, :, :])
                        elif gname == "k":
                            nc.scalar.copy(out=kb[:, i, 2 * half:2 * half + 2, :],
                                           in_=gps[:, :, :])
                        else:
                            nc.scalar.copy(out=vb[:, i, 2 * half:2 * half + 2, :],
                                           in_=gps[:, :, :])

                # ---- stage C: scores + softmax (i > 0) ----
                if i > 0:
                    nj = i + 1
                    scpt = ptin.tile([128, tb], f32, tag="pt")
                    scpt2 = (ptin.tile([128, tb], f32, name="scpt2", tag="pt")
                             if nj > 3 else None)
                    def _sc(j):
                        return (scpt[32 * j:32 * j + 8, :] if j < 3
                                else scpt2[0:8, :])
                    for c in range(4):
                        pmul = sc16.tile([128, 4, tb], bf16, tag="sc16")
                        nc.vector.tensor_mul(pmul[:, :nj, :],
                                             _bc(qb[:, c, :], 0, nj),
                                             kb[:, 0:nj, c, :])
                        for j in range(nj):
                            nc.tensor.matmul(_sc(j), bd_sb[:, c, :],
                                             pmul[:, j, :],
                                             start=(c == 0), stop=(c == 3),
                                             skip_group_check=True)
                    attn = sm.tile([8, 4, tb], f32r, tag="attn")
                    for j in range(nj):
                        nc.scalar.copy(out=attn[:, j, :], in_=_sc(j))
                    mx = sm.tile([8, tb], f32, tag="mx")
                    nc.vector.tensor_reduce(
                        out=mx[:, :], in_=attn[:, 0:nj, :].rearrange("p j t -> p t j"),
                        axis=mybir.AxisListType.X, op=mybir.AluOpType.max)
                    nc.vector.tensor_sub(attn[:, 0:nj, :], attn[:, 0:nj, :],
                                         _bc(mx[:, :], 0, nj))
                    nc.scalar.activation(out=attn[:, 0:nj, :], in_=attn[:, 0:nj, :],
                                         func=mybir.ActivationFunctionType.Exp)
                    smsum = sm.tile([8, tb], f32, tag="smsum")
                    nc.vector.tensor_reduce(
                        out=smsum[:, :],
                        in_=attn[:, 0:nj, :].rearrange("p j t -> p t j"),
                        axis=mybir.AxisListType.X, op=mybir.AluOpType.add)
                    nc.scalar.activation(out=smsum[:, :], in_=smsum[:, :],
                                         func=mybir.ActivationFunctionType.Ln)
                    nc.scalar.activation(out=smsum[:, :], in_=smsum[:, :],
                                         func=mybir.ActivationFunctionType.Exp,
                                         scale=-1.0)
                    nc.vector.tensor_mul(attn[:, 0:nj, :], attn[:, 0:nj, :],
                                         _bc(smsum[:, :], 0, nj))

                # ---- stage D: ctx + Wo ----
                if i == 0:
                    ctxt = zc.tile([128, 4, tb], f32r, tag="zc")
                    nc.gpsimd.tensor_copy(ctxt[:, :, :], vb[:, 0, :, :])
                else:
                    nj = i + 1
                    ctxt = zc.tile([128, 4, tb], f32r, tag="zc")
                    for c in range(4):
                        npair = (nj + 1) // 2
                        abt = []
                        for half in range(npair):
                            at = pbig.tile([128, 2, tb], f32, tag="pb")
                            for jj in range(2):
                                j = half * 2 + jj
                                if j < nj:
                                    nc.tensor.matmul(at[:, jj, :],
                                                     ebc_sb[:, c, :],
                                                     attn[:, j, :],
                                                     start=True, stop=True)
                            abt.append(at)
                        tmp = tm.tile([128, 4, tb], f32, tag="tmp32")
                        for half in range(npair):
                            jn = min(2, nj - half * 2)
                            nc.vector.tensor_mul(
                                tmp[:, half * 2:half * 2 + jn, :],
                                abt[half][:, 0:jn, :],
                                vb[:, half * 2:half * 2 + jn, c, :])
                        nc.gpsimd.tensor_add(ctxt[:, c, :], tmp[:, 0, :],
                                             tmp[:, 1, :])
                        for j in range(2, nj):
                            nc.gpsimd.tensor_add(ctxt[:, c, :], ctxt[:, c, :],
                                                 tmp[:, j, :])

                r_lo = pbig.tile([128, 2, tb], f32, tag="pb")
                r_hi = pbig.tile([128, 2, tb], f32, tag="pb")
                for m in range(4):
                    ps = (r_lo, r_hi)[m // 2]
                    for c in range(4):
                        nc.tensor.matmul(ps[:, m % 2, :],
                                         wo_sb[:, c, m * 128:(m + 1) * 128],
                                         ctxt[:, c, :], start=(c == 0),
                                         stop=(c == 3 and not o_bias))
                    if o_bias:
                        nc.tensor.matmul(ps[:, m % 2, :],
                                         bo_sb[0:1, m * 128:(m + 1) * 128],
                                         ones_row[:, :], start=False, stop=True)
                # stacked' = S + refined  (in place on s_sb)
                nc.vector.tensor_add(s_sb[:, 0:2, :], s_sb[:, 0:2, :], r_lo[:, :, :])
                nc.vector.tensor_add(s_sb[:, 2:4, :], s_sb[:, 2:4, :], r_hi[:, :, :])

                # ---- stage E: output projection + x residual ----
                mch = DPAD[i] // 128
                for mp in range(0, mch, 2):
                    mn = min(2, mch - mp)
                    ops = pbig.tile([128, 2, tb], f32, tag="pb")
                    xres = xr.tile([128, 2, tb], f32r, tag="xres")
                    src = xT[i][128 * mp:128 * (mp + mn), t0:t0 + tb].rearrange(
                        "(c p) t -> p c t", p=128)
                    nc.sync.dma_start(out=xres[:, 0:mn, :], in_=src)
                    for mi in range(mn):
                        m = mp + mi
                        for c in range(4):
                            nc.tensor.matmul(ops[:, mi, :],
                                             wout_sb[i][:, c, m * 128:(m + 1) * 128],
                                             s_sb[:, c, :],
                                             start=(c == 0), stop=False)
                        nc.tensor.matmul(ops[:, mi, :], i128_sb[:, :],
                                         xres[:, mi, :], start=False, stop=True)
                    for mi in range(mn):
                        m = mp + mi
                        rows = min(128, EXPERT_DIMS[i] - 128 * m)
                        if rows <= 0:
                            continue
                        osb = op_.tile([128, tb], f32, tag="osb")
                        nc.scalar.copy(out=osb[:, :], in_=ops[:, mi, :])
                        nc.sync.dma_start(
                            out=outT[i][128 * m:128 * m + rows, t0:t0 + tb],
                            in_=osb[:rows, :])
    nc.compile()
    return nc


def xT_rearr(handle, kc, m):
    return handle[:, :].rearrange("(c p) m -> p c m", p=128)


def prep_shared(inputs):
    """Host-side preprocessing of weights/constants (shared by all cores)."""
    f32 = np.float32
    g = np.asarray(inputs["ln_g"], f32)
    beta = np.asarray(inputs["ln_b"], f32)
    Wqkv = np.asarray(inputs["Wqkv"], f32)
    bqkv = np.asarray(inputs["bqkv"], f32)
    Wo = np.asarray(inputs["Wo"], f32)
    bo = np.asarray(inputs["bo"], f32)

    Wq_eff = Wqkv * g[None, :]
    bq_eff = bqkv + Wqkv @ beta
    Wq_eff[:D] *= 1.0 / np.sqrt(HD)
    bq_eff[:D] *= 1.0 / np.sqrt(HD)

    out = {}
    for i in range(NE):
        d, dp = EXPERT_DIMS[i], DPAD[i]
        Win = np.asarray(inputs[f"Win{i}"], f32)          # (D, d)
        w = np.zeros((dp, 514), f32)
        w[:d, :D] = Win.T
        w[:d, 512] = Win.mean(axis=0)
        out[f"winT{i}"] = w
        Wout = np.asarray(inputs[f"Wout{i}"], f32)        # (d, D)
        wo_ = np.zeros((D, dp), f32)
        wo_[:, :d] = Wout.T
        out[f"woutT{i}"] = wo_
    out["wqkvT"] = np.ascontiguousarray(Wq_eff.T)         # (D, 3D)
    out["woT"] = np.ascontiguousarray(Wo.T)
    out["i128"] = np.eye(128, dtype=f32)
    bd = np.zeros((128, 4, 8), f32)
    for c in range(4):
        for p in range(128):
            bd[p, c, 2 * c + p // 64] = 1.0
    out["bd"] = bd.reshape(128, 32).astype(BF)
    ebc = np.zeros((8, 4, 128), f32)
    for c in range(4):
        for p in range(128):
            ebc[2 * c + p // 64, c, p] = 1.0
    out["ebc"] = np.ascontiguousarray(ebc.reshape(8, 512))
    out["onesm"] = np.full((128, 1), 1.0 / 512.0, f32).astype(BF)
    out["onesk"] = np.ones((1, 128), f32)
    qkv_bias = bool(np.abs(bq_eff).max() > 0)
    o_bias = bool(np.abs(bo).max() > 0)
    if qkv_bias:
        out["bqkv_eff"] = bq_eff.reshape(1, 3 * D)
    if o_bias:
        out["bo_eff"] = bo.reshape(1, D)
    return out, qkv_bias, o_bias


def prep_core_inputs(inputs, core, shared, tcore=T):
    """Per-core inputs: transposed, padded x slices."""
    m = dict(shared)
    for i in range(NE):
        d, dp = EXPERT_DIMS[i], DPAD[i]
        x = np.asarray(inputs[f"x{i}"], np.float32)[core]      # (T, d)
        xt = np.zeros((dp, tcore), np.float32)
        xt[:d, :] = x.T
        m[f"xT{i}"] = xt
    return m


_cached = {}


def kernel(**inputs):
    shared, qkv_bias, o_bias = prep_shared(inputs)
    key = (T, 512, qkv_bias, o_bias)
    if key not in _cached:
        _cached[key] = build_program(T, 512, qkv_bias, o_bias)
    nc = _cached[key]
    in_maps = [prep_core_inputs(inputs, c, shared) for c in range(NCORES)]
    res = run_bass_kernel_spmd(nc, in_maps, list(range(NCORES)))
    outs = []
    for i in range(NE):
        full = np.empty((B, T, EXPERT_DIMS[i]), np.float32)
        for c in range(NCORES):
            full[c] = res.results[c][f"outT{i}"].T
        outs.append(full)
    return tuple(outs)
